# revision 3
# baseline (speedup 1.0000x reference)
"""Banded causal self-attention (B=2, T=2048, C=1024, 16 heads, band=256) on 8 NeuronCores.

Sharding: (batch, T) split into 8 chunks of 512 queries; each core gets its
query chunk plus a 256-key halo, full weights, and produces its 512 rows of
the final output directly (no collectives).

Layout trick: host passes x.T and W.T so the device needs no data transposes.
 - Q^T, K^T computed as [o, t] tiles (lhsT = W.T tile, rhs = x.T tile)
 - V computed natural [t, o] (lhsT = x.T tile, rhs = Wv.T tile), stored
   interleaved per head with a ones-column: [t, 16*(64+1)]
 - S^T[k, q] per (head, key-tile) via 64-contraction quadrant-packed matmuls
   (head pair shares the PE array at tile rows 0-63 / 64-127)
 - softmax: exp(scale*S) with no max subtraction (scores bounded), then
   multiplicative {0,1} band/causal masks, denominators via the fused
   ones-column in the attV matmul, normalize via reciprocal + gpsimd
   partition-broadcast
 - output projection consumes the attention output's natural [c', t] layout
   (lhsT = yattT tile, rhs = Wp.T tile) and adds bp + bv@Wp.T (host-folded)
All matmul operands are float32r (full PE rate, ~1e-4 relative error).
"""
import sys
sys.path.insert(0, '/opt/trn_rl_repo')
import numpy as np

B, T, C = 2, 2048, 1024
NH, HD = 16, 64
BAND = 256
NCORES = 8
TQ = 512            # queries per core
TK = 768            # keys per core (incl. 256 halo)
QT_TILES = 4        # query tiles of 128
KG_TILES = 6        # key tiles of 128
SCALE = 1.0 / np.sqrt(HD)

# per key-tile kg: (first query-tile of the padded span, span width in tiles)
SPAN_S = [0, 0, 0, 1, 2, 2]
SPAN_W = [2, 2, 3, 3, 2, 2]
# column ranges of the span that need a mask multiply (others are always-valid)
MASK_MUL = {
    0: [(0, 256)],
    1: [(0, 256)],
    2: [(0, 128), (256, 384)],
    3: [(0, 128), (256, 384)],
    4: [(0, 128)],
    5: [(0, 256)],
}

_NC = None


def _build():
    import concourse.bacc as bacc
    import concourse.tile as tile
    from concourse import mybir

    f32 = mybir.dt.float32
    f32r = mybir.dt.float32r
    Exp = mybir.ActivationFunctionType.Exp

    nc = bacc.Bacc()
    xt_d = nc.dram_tensor("xt", [C, TK], f32r, kind="ExternalInput")
    wqt_d = nc.dram_tensor("wqt", [C, C], f32r, kind="ExternalInput")
    wkt_d = nc.dram_tensor("wkt", [C, C], f32r, kind="ExternalInput")
    wvt_d = nc.dram_tensor("wvt", [C, C], f32r, kind="ExternalInput")
    wpt_d = nc.dram_tensor("wpt", [C, C], f32r, kind="ExternalInput")
    mask_d = nc.dram_tensor("mask", [KG_TILES, 128, 384], f32r, kind="ExternalInput")
    bq_d = nc.dram_tensor("bqr", [128, 8], f32, kind="ExternalInput")
    bk_d = nc.dram_tensor("bkr", [128, 8], f32, kind="ExternalInput")
    bfold_d = nc.dram_tensor("bfold", [128, C], f32, kind="ExternalInput")
    ones_d = nc.dram_tensor("onesc", [128, NH], f32r, kind="ExternalInput")
    y_d = nc.dram_tensor("y", [TQ, C], f32, kind="ExternalOutput")

    with tile.TileContext(nc) as tc:
        with tc.tile_pool(name="xt", bufs=8) as xtp, \
             tc.tile_pool(name="cst", bufs=1) as cst, \
             tc.tile_pool(name="qt", bufs=8) as qtp, \
             tc.tile_pool(name="kt", bufs=8) as ktp, \
             tc.tile_pool(name="vt", bufs=6) as vtp, \
             tc.tile_pool(name="yat", bufs=8) as yatp, \
             tc.tile_pool(name="w", bufs=12) as wp, \
             tc.tile_pool(name="pt", bufs=6) as ptp, \
             tc.tile_pool(name="sml", bufs=4) as sml, \
             tc.tile_pool(name="yo", bufs=3) as yop:

            # ---- resident inputs ----
            mask_sb = cst.tile([128, KG_TILES, 384], f32r, tag="mask")
            nc.sync.dma_start(mask_sb[:], mask_d[:].transpose([1, 0, 2]))
            bq_sb = cst.tile([128, 8], f32, tag="bq")
            bk_sb = cst.tile([128, 8], f32, tag="bk")
            bfold_sb = cst.tile([128, C], f32, tag="bfold")
            nc.sync.dma_start(bq_sb[:], bq_d[:])
            nc.sync.dma_start(bk_sb[:], bk_d[:])
            nc.sync.dma_start(bfold_sb[:], bfold_d[:])
            xts = []
            for cb in range(8):
                xt = xtp.tile([128, TK], f32r, tag="xt")
                nc.sync.dma_start(xt[:], xt_d[cb * 128:(cb + 1) * 128, :])
                xts.append(xt)

            QTs = [qtp.tile([128, TQ], f32r, tag="qt", name=f"QT{i}") for i in range(8)]
            KTs = [ktp.tile([128, TK], f32r, tag="kt", name=f"KT{i}") for i in range(8)]
            Vs = [vtp.tile([128, NH, HD + 1], f32r, tag="v", name=f"V{i}") for i in range(KG_TILES)]
            YATs = [yatp.tile([128, TQ], f32r, tag="yat", name=f"YAT{i}") for i in range(8)]

            # ---- phase 1: Q^T and K^T projections ----
            with tc.tile_pool(name="ppj", bufs=2, space="PSUM") as pp:
                wq = []
                for cb in range(8):
                    wt = wp.tile([128, C], f32r, tag="w")
                    nc.sync.dma_start(wt[:], wqt_d[cb * 128:(cb + 1) * 128, :])
                    wq.append(wt)
                for ob in range(8):
                    ps = pp.tile([128, 512], f32, tag="pp")
                    for cb in range(8):
                        nc.tensor.matmul(ps[:, :], wq[cb][:, ob * 128:(ob + 1) * 128],
                                         xts[cb][:, 256:768],
                                         start=(cb == 0), stop=(cb == 7))
                    nc.scalar.add(QTs[ob][:, :], ps[:, :], bq_sb[:, ob:ob + 1])

                wk = []
                for cb in range(8):
                    wt = wp.tile([128, C], f32r, tag="w")
                    nc.sync.dma_start(wt[:], wkt_d[cb * 128:(cb + 1) * 128, :])
                    wk.append(wt)
                for ob in range(8):
                    for (t0, t1) in ((0, 512), (512, 768)):
                        ps = pp.tile([128, 512], f32, tag="pp")
                        for cb in range(8):
                            nc.tensor.matmul(ps[:, 0:t1 - t0],
                                             wk[cb][:, ob * 128:(ob + 1) * 128],
                                             xts[cb][:, t0:t1],
                                             start=(cb == 0), stop=(cb == 7))
                        nc.scalar.add(KTs[ob][:, t0:t1], ps[:, 0:t1 - t0],
                                      bk_sb[:, ob:ob + 1])

                # ---- phase 2: V projection (natural layout, 65-interleaved) ----
                wv = []
                for cb in range(8):
                    wt = wp.tile([128, C], f32r, tag="w")
                    nc.sync.dma_start(wt[:], wvt_d[cb * 128:(cb + 1) * 128, :])
                    wv.append(wt)
                for tt in range(KG_TILES):
                    for oc in range(2):
                        ps = pp.tile([128, 512], f32, tag="pp")
                        for cb in range(8):
                            nc.tensor.matmul(ps[:, :],
                                             xts[cb][:, tt * 128:(tt + 1) * 128],
                                             wv[cb][:, oc * 512:(oc + 1) * 512],
                                             start=(cb == 0), stop=(cb == 7))
                        dst = Vs[tt][:, oc * 8:(oc + 1) * 8, 0:HD]
                        src = ps[:, :].rearrange("p (h d) -> p h d", d=HD)
                        nc.vector.tensor_copy(dst, src)
                    nc.sync.dma_start(Vs[tt][:, :, HD], ones_d[:, :])

            # ---- phase 3: banded attention per head pair ----
            with tc.tile_pool(name="pss", bufs=2, space="PSUM") as pss, \
                 tc.tile_pool(name="pso", bufs=2, space="PSUM") as pso:
                for hp in range(8):
                    psO = [pso.tile([65, TQ], f32, tag=f"o{i}", name=f"psO{hp}_{i}") for i in range(2)]
                    for kg in range(KG_TILES):
                        s, w = SPAN_S[kg], SPAN_W[kg]
                        N = w * 128
                        for i in range(2):
                            r0, r1 = (0, 64) if i == 0 else (64, 128)
                            h = 2 * hp + i
                            psS = pss.tile([128, 384], f32, tag=f"s{i}")
                            nc.tensor.matmul(psS[:, 0:N],
                                             KTs[hp][r0:r1, kg * 128:(kg + 1) * 128],
                                             QTs[hp][r0:r1, s * 128:s * 128 + N],
                                             start=True, stop=True)
                            pt = ptp.tile([128, 384], f32r, tag="pt")
                            nc.scalar.activation(pt[:, 0:N], psS[:, 0:N], Exp,
                                                 scale=float(SCALE))
                            for (c0, c1) in MASK_MUL[kg]:
                                nc.vector.tensor_mul(pt[:, c0:c1], pt[:, c0:c1],
                                                     mask_sb[:, kg, c0:c1])
                            nc.tensor.matmul(psO[i][0:65, s * 128:s * 128 + N],
                                             Vs[kg][:, h, 0:HD + 1],
                                             pt[:, 0:N],
                                             start=(kg == 0), stop=(kg == 5))
                    for i in range(2):
                        rl = sml.tile([1, TQ], f32, tag="rl")
                        nc.vector.reciprocal(rl[:, :], psO[i][64:65, :])
                        rb = sml.tile([64, TQ], f32, tag="rb")
                        nc.gpsimd.partition_broadcast(rb[:, :], rl[:, :])
                        nc.vector.tensor_mul(YATs[hp][i * 64:(i + 1) * 64, :],
                                             psO[i][0:64, :], rb[:, :])

            # ---- phase 4: output projection ----
            with tc.tile_pool(name="pyo", bufs=2, space="PSUM") as pyo:
                wpt = []
                for cb in range(8):
                    wt = wp.tile([128, C], f32r, tag="w")
                    nc.sync.dma_start(wt[:], wpt_d[cb * 128:(cb + 1) * 128, :])
                    wpt.append(wt)
                for tt in range(QT_TILES):
                    for oc in range(2):
                        ps = pyo.tile([128, 512], f32, tag="py")
                        for cb in range(8):
                            nc.tensor.matmul(ps[:, :],
                                             YATs[cb][:, tt * 128:(tt + 1) * 128],
                                             wpt[cb][:, oc * 512:(oc + 1) * 512],
                                             start=(cb == 0), stop=(cb == 7))
                        yo = yop.tile([128, 512], f32, tag="yo")
                        nc.vector.tensor_add(yo[:, :], ps[:, :],
                                             bfold_sb[:, oc * 512:(oc + 1) * 512])
                        nc.sync.dma_start(
                            y_d[tt * 128:(tt + 1) * 128, oc * 512:(oc + 1) * 512],
                            yo[:, :])

    nc.finalize()
    return nc


def _host_inputs(x, Wq, bq, Wk, bk, Wv, bv, Wp, bp):
    x = np.ascontiguousarray(np.asarray(x, dtype=np.float32))
    Wq, Wk, Wv, Wp = (np.asarray(w, dtype=np.float32) for w in (Wq, Wk, Wv, Wp))
    bq, bk, bv, bp = (np.asarray(b_, dtype=np.float32) for b_ in (bq, bk, bv, bp))

    wqt = np.ascontiguousarray(Wq.T)
    wkt = np.ascontiguousarray(Wk.T)
    wvt = np.ascontiguousarray(Wv.T)
    wpt = np.ascontiguousarray(Wp.T)
    bqr = np.ascontiguousarray(bq.reshape(8, 128).T)
    bkr = np.ascontiguousarray(bk.reshape(8, 128).T)
    bfold = (bp + bv @ Wp.T).astype(np.float32)
    bfold_t = np.ascontiguousarray(np.broadcast_to(bfold[None, :], (128, C)))

    in_maps = []
    for core in range(NCORES):
        b, j = divmod(core, 4)
        qs = j * TQ
        xt = np.zeros((C, TK), dtype=np.float32)
        if j == 0:
            xt[:, 256:768] = x[b, 0:TQ].T
        else:
            xt[:, :] = x[b, qs - 256:qs + TQ].T

        mask = np.zeros((KG_TILES, 128, 384), dtype=np.float32)
        r = np.arange(128)
        for kg in range(KG_TILES):
            for slot in range(SPAN_W[kg]):
                qt = SPAN_S[kg] + slot
                if qt < kg - 2 or qt > kg:
                    continue  # outside attend window: stays 0
                i = qs + qt * 128 + r[None, :]
                jj = qs - 256 + kg * 128 + r[:, None]
                m = (jj <= i) & (jj >= i - BAND) & (jj >= 0)
                mask[kg][:, slot * 128:(slot + 1) * 128] = m

        in_maps.append({
            "xt": np.ascontiguousarray(xt),
            "wqt": wqt, "wkt": wkt, "wvt": wvt, "wpt": wpt,
            "mask": mask,
            "bqr": bqr, "bkr": bkr,
            "bfold": bfold_t,
            "onesc": np.ones((128, NH), dtype=np.float32),
        })
    return in_maps


def kernel(x, Wq, bq, Wk, bk, Wv, bv, Wp, bp):
    global _NC
    from concourse.bass_utils import run_bass_kernel_spmd

    if _NC is None:
        _NC = _build()
    in_maps = _host_inputs(x, Wq, bq, Wk, bk, Wv, bv, Wp, bp)
    res = run_bass_kernel_spmd(_NC, in_maps, core_ids=list(range(NCORES)))
    out = np.empty((B, T, C), dtype=np.float32)
    for core in range(NCORES):
        b, j = divmod(core, 4)
        out[b, j * TQ:(j + 1) * TQ, :] = res.results[core]["y"]
    return out
